# revision 20
# baseline (speedup 1.0000x reference)
"""ConvDU (spatial propagation) Trainium2 Bass kernel.

Reference semantics (per batch element):
    forward scan over rows i = 1..h-1:
        full[i] = relu(conv1x9(full[i-1]) + bias) + fea[i]      (full[0] = fea[0])
    backward scan over rows i = h-2..1:
        out[i]  = relu(conv1x9(out[i+1]) + bias) + full[i]
    out[0] = full[0], out[h-1] = full[h-1]

Sharding: data-parallel over batch n=8 -> one batch element per NeuronCore.
Per core the whole [256, 128, 128] fp32 plane lives in SBUF; the scan is a
strictly sequential chain of 253 steps, each a 9-tap 256->256 channel GEMM:
2 out-chunks x 2 in-chunks x 9 shifted taps = 36 matmuls/step accumulated in
PSUM (zero-padding at the width edges falls out of partial-width matmuls).
The per-step critical path is one fused DVE op per chunk, using
    relu(ps + b) + row  ==  (ps max -b) + (b + row)
with brow = b + row precomputed one step ahead on the otherwise idle Scalar
engine. The two PSUM groups' matmuls are interleaved so chunk 0 retires ~14
matmuls before the step boundary, hiding that DVE op entirely: the steady
state runs at the PE's LDWEIGHTS/issue floor (~55ns per matmul, ~2.0us per
step, zero boundary stalls).
Matmul operands are fp16 (PSUM accumulation is fp32): empirically the fp16
operand quantization gives ~2e-4 relative error over the whole double scan
while running the PE at full speed (fp32 matmul is 4x slower). The fp32
residual stream is kept exactly; a small fp16 "mirror" of the latest row
feeds the next step's matmuls.
"""

import numpy as np

N_CORES = 8
C = 256
H = 128
W = 128
K = 9
PAD = 4
P = 128
CH = C // P  # channel chunks of 128

_NC_CACHE = {}


def _build_nc(h=H):
    import concourse.bacc as bacc
    import concourse.mybir as mybir
    import concourse.tile as tile

    dt = mybir.dt
    nc = bacc.Bacc("TRN2", target_bir_lowering=False, debug=False)
    fea_d = nc.dram_tensor("fea", [CH, P, h * W], dt.float32, kind="ExternalInput")
    wT_d = nc.dram_tensor("wT", [P, CH, K, CH, P], dt.float16, kind="ExternalInput")
    bias_d = nc.dram_tensor("bias", [P, CH], dt.float32, kind="ExternalInput")
    out_d = nc.dram_tensor("out", [CH, P, h * W], dt.float32, kind="ExternalOutput")

    with tile.TileContext(nc) as tc:
        _convdu(tc, nc, fea_d.ap(), wT_d.ap(), bias_d.ap(), out_d.ap(), h, mybir)
    nc.compile()
    return nc


def _convdu(tc, nc, fea, wT, bias, out, h, mybir):
    from contextlib import ExitStack

    dt = mybir.dt
    f32, f16 = dt.float32, dt.float16
    Amax, Aadd = mybir.AluOpType.max, mybir.AluOpType.add
    BLK = 16 if h % 16 == 0 else h
    nblk = h // BLK

    # open each PSUM accumulation group with the full-width center tap
    TAPS0 = [PAD] + [k for k in range(K) if k != PAD]

    with ExitStack() as ctx:
        const = ctx.enter_context(tc.tile_pool(name="const", bufs=1))
        planes = ctx.enter_context(tc.tile_pool(name="planes", bufs=1))
        psum = ctx.enter_context(tc.tile_pool(name="psum", bufs=6, space="PSUM"))

        plane = [
            planes.tile([P, h * W], f32, tag=f"plane{c}", name=f"plane{c}")
            for c in range(CH)
        ]
        # load order mirrors first-use order: bias (tiny; gates negb/brow),
        # rows 0-1 (mirror init + first brow), weights (i2=0 feeds the first
        # 13 matmuls), rest of block 0, remaining blocks
        bsb = const.tile([P, CH], f32)
        nc.sync.dma_start(bsb[:], bias)
        for c2 in range(CH):
            nc.sync.dma_start(plane[c2][:, 0:W], fea[c2, :, 0:W])
        for c2 in range(CH):
            nc.sync.dma_start(plane[c2][:, W : 2 * W], fea[c2, :, W : 2 * W])
        wsb = const.tile([P, CH, K, CH, P], f16)
        for i2 in range(CH):
            for o2 in range(CH):
                nc.sync.dma_start(wsb[:, i2, :, o2, :], wT[:, i2, :, o2, :])
        for c2 in range(CH):
            nc.sync.dma_start(
                plane[c2][:, 2 * W : BLK * W], fea[c2, :, 2 * W : BLK * W]
            )
        for b in range(1, nblk):
            for c2 in range(CH):
                nc.sync.dma_start(
                    plane[c2][:, b * BLK * W : (b + 1) * BLK * W],
                    fea[c2, :, b * BLK * W : (b + 1) * BLK * W],
                )
        # fp16 mirror of the latest scan row, double-buffered by step parity
        mir = const.tile([P, 2, CH, W], f16)
        negb = const.tile([P, CH], f32)

        # PE warmup: dummy matmuls on a zeroed tile so HAM un-throttles
        # (K 4/8 -> 8/8) before the first real matmul, while DMAs land.
        dummy = const.tile([P, W], f16)
        nc.vector.memset(dummy[:], 0.0)
        dps = psum.tile([P, W], f32, tag="ps")
        for _ in range(160):
            nc.tensor.matmul(dps[:], dummy[:], dummy[:], start=True, stop=True)

        nc.vector.tensor_scalar_mul(negb[:], bsb[:], -1.0)
        # mirror slot 0 <- fp16(row 0)
        for c2 in range(CH):
            nc.vector.tensor_copy(mir[:, 0, c2, :], plane[c2][:, 0:W])

        def mk_group(ps, ssrc, o2):
            mms = []
            for i2 in range(CH):
                for j, k in enumerate(TAPS0 if i2 == 0 else range(K)):
                    s = k - PAD
                    lo = max(0, -s)
                    hi = W - max(0, s)
                    mms.append(
                        (
                            ps[:, lo:hi],
                            wsb[:, i2, k, o2, :],
                            mir[:, ssrc, i2, lo + s : hi + s],
                            i2 == 0 and j == 0,
                            i2 == CH - 1 and k == K - 1,
                        )
                    )
            return mms

        # brow[slot][o2] = bias + next step's residual row; computed one step
        # ahead on the Scalar engine, so the critical epilogue is a single
        # DVE op:  relu(ps+b)+row  ==  max(ps,-b) + (b+row)  ==
        # (ps max negb) add brow.
        brow = const.tile([P, 2, CH, W], f32)

        def prep_brow(t, row):
            sl = t % 2
            for o2 in range(CH):
                nc.scalar.add(
                    brow[:, sl, o2, :],
                    plane[o2][:, row * W : (row + 1) * W],
                    bsb[:, o2 : o2 + 1],
                )

        def step(t, row_dst, next_row=None):
            ssrc, sdst = (t - 1) % 2, t % 2
            psA = psum.tile([P, W], f32, tag="ps")
            psB = psum.tile([P, W], f32, tag="ps")
            A = mk_group(psA, ssrc, 0)
            B = mk_group(psB, ssrc, 1)
            # Interleave so group A retires ~14 MMs before the step ends: its
            # single-op epilogue then overlaps B's remaining matmuls instead
            # of stalling the next step's opening matmul.
            order = A[0:9] + B[0:4] + A[9:18] + B[4:18]
            for ps_, lhsT, rhs, st, sp in order:
                nc.tensor.matmul(ps_, lhsT, rhs, start=st, stop=sp)
            if t < 2 * h - 3:  # the last step feeds no next step
                for o2, ps_ in ((0, psA), (1, psB)):
                    # next-step conv input (fp16, on the critical path)
                    nc.vector.scalar_tensor_tensor(
                        mir[:, sdst, o2, :],
                        ps_[:],
                        negb[:, o2 : o2 + 1],
                        brow[:, t % 2, o2, :],
                        Amax,
                        Aadd,
                    )
            if next_row is not None:
                prep_brow(t + 1, next_row)
            for o2, ps_ in ((0, psA), (1, psB)):
                # true fp32 output row (lags; off the critical path)
                nc.vector.scalar_tensor_tensor(
                    plane[o2][:, row_dst * W : (row_dst + 1) * W],
                    ps_[:],
                    negb[:, o2 : o2 + 1],
                    brow[:, t % 2, o2, :],
                    Amax,
                    Aadd,
                )

        OB = min(8, BLK)  # output dma granularity (rows)

        def dma_out_rows(r0, r1):
            for c2 in range(CH):
                nc.sync.dma_start(
                    out[c2, :, r0 * W : r1 * W], plane[c2][:, r0 * W : r1 * W]
                )

        prep_brow(1, 1)
        for t in range(1, h):  # forward: writes row t
            step(t, t, (t + 1) if t < h - 1 else h - 2)
        for t in range(h, 2 * h - 2):  # backward: writes row 2h-2-t
            r = 2 * h - 2 - t
            step(t, r, (r - 1) if t < 2 * h - 3 else None)
            if r % OB == 0 and r > 0:
                dma_out_rows(r, min(r + OB, h))
            elif r == OB // 2:
                dma_out_rows(r, OB)
        dma_out_rows(0, OB // 2)


def _prep_static(weight, bias):
    # wT[i, i2, k, o2, o] = weight[o2*128+o, i2*128+i, k], fp16
    w = np.asarray(weight, dtype=np.float32).reshape(CH, P, CH, P, K)
    wT = np.ascontiguousarray(w.transpose(3, 2, 4, 0, 1)).astype(np.float16)
    # bias32[i, o2] = bias[o2*128+i]
    b32 = np.ascontiguousarray(
        np.asarray(bias, dtype=np.float32).reshape(CH, P).T
    )
    return wT, b32


def run(fea, weight, bias, trace=False, **spmd_kwargs):
    """Returns (output [n,C,H,W] fp32, BassKernelResults)."""
    from concourse.bass_utils import run_bass_kernel_spmd

    fea = np.asarray(fea, dtype=np.float32)
    n = fea.shape[0]
    assert fea.shape == (n, C, H, W)
    wT, b16 = _prep_static(weight, bias)
    in_maps = []
    for bi in range(n):
        feab = np.ascontiguousarray(fea[bi].reshape(CH, P, H * W))
        in_maps.append({"fea": feab, "wT": wT, "bias": b16})
    if H not in _NC_CACHE:
        _NC_CACHE[H] = _build_nc(H)
    nc = _NC_CACHE[H]
    try:
        res = run_bass_kernel_spmd(
            nc, in_maps, core_ids=list(range(n)), trace=trace, **spmd_kwargs
        )
    except Exception:
        # transient device faults (e.g. NRT_EXEC_UNIT_UNRECOVERABLE) recover
        # on re-execution; the compiled NEFF is reused
        res = run_bass_kernel_spmd(
            nc, in_maps, core_ids=list(range(n)), trace=trace, **spmd_kwargs
        )
    outs = [res.results[bi]["out"].reshape(C, H, W) for bi in range(n)]
    return np.stack(outs, axis=0).astype(np.float32), res


def kernel(fea, weight, bias):
    out, _ = run(fea, weight, bias, trace=False)
    return out
